# revision 1
# baseline (speedup 1.0000x reference)
"""Gated causal attention (B=2, L=2048, HID=2048, NH=16, HD=128) on 8 trn2 cores.

Sharding: data-parallel over batch (cores 0-3 batch 0, cores 4-7 batch 1) x
tensor-parallel over heads (4 heads per core within its batch). Each core:
  - projects q/k/v/g for its 4 heads (fp32r matmuls, x.T resident in SBUF)
  - RoPE on q/k in [d, m] layout (rotate-half via SBUF->SBUF swap DMA)
  - causal attention per head in S_T = [kpos, q] layout; softmax denominators
    via an all-ones stationary matmul; no max-subtraction (scores are small)
  - per-head RMSNorm + silu gating on broadcast [128, m] tiles
  - o_proj partial [L, 2048]
Host sums the 4 partials per batch and stacks the two batches.
"""

import numpy as np

B, L, HID, NH, HD = 2, 2048, 2048, 16, 128
EPS = 1e-5
SCALE = HD ** -0.5
ROPE_BASE = 10000.0
NCORES = 8
HPC = 4            # heads per core
NDIM = HPC * HD    # 512 projection dims per core
P = 128
KC = HID // P      # 16 k-chunks
CC = L // P        # 16 kpos chunks
QT = 512           # q tile (fp32r moving max)
NHALF = L // 2     # AV/den psum half width
NCH = (4 * NDIM) // P  # 16 fused projection n-chunks (q|k|v|g)


def _build(nc, mybir, tile):
    from contextlib import ExitStack

    f32 = mybir.dt.float32
    f32r = mybir.dt.float32r
    AF = mybir.ActivationFunctionType
    OP = mybir.AluOpType

    xT = nc.dram_tensor("xT", [HID, L], f32r, kind="ExternalInput")
    # wT blocked: [k-chunk, n-chunk, 128, 128]; n order = q|k|v|g, each 512
    wTb = nc.dram_tensor("wTb", [KC, NCH, P, P], f32r, kind="ExternalInput")
    woT = nc.dram_tensor("woT", [NDIM, HID], f32r, kind="ExternalInput")
    cosq = nc.dram_tensor("cosq", [P, L], f32, kind="ExternalInput")
    ssinq = nc.dram_tensor("ssinq", [P, L], f32, kind="ExternalInput")
    cosk = nc.dram_tensor("cosk", [P, L], f32, kind="ExternalInput")
    ssink = nc.dram_tensor("ssink", [P, L], f32, kind="ExternalInput")
    ones_t = nc.dram_tensor("ones_t", [P, P], f32r, kind="ExternalInput")
    oneshd_t = nc.dram_tensor("oneshd_t", [P, P], f32r, kind="ExternalInput")
    ident_t = nc.dram_tensor("ident_t", [P, P], f32r, kind="ExternalInput")
    masks_t = nc.dram_tensor("masks_t", [4, P, QT], f32r, kind="ExternalInput")
    nw_t = nc.dram_tensor("nw_t", [P, 1], f32, kind="ExternalInput")
    out_partial = nc.dram_tensor("out_partial", [L, HID], f32,
                                 kind="ExternalOutput")

    with tile.TileContext(nc) as tc, ExitStack() as octx:
        const = octx.enter_context(tc.tile_pool(name="const", bufs=1))
        ones = const.tile([P, P], f32r, tag="ones")
        oneshd = const.tile([P, P], f32r, tag="oneshd")
        ident = const.tile([P, P], f32r, tag="ident")
        nw = const.tile([P, 1], f32, tag="nw")
        masks = [const.tile([P, QT], f32r, tag=f"mask{r}", name=f"mask{r}") for r in range(4)]

        # DRAM staging pools (tracked by Tile)
        dstage = octx.enter_context(tc.tile_pool(name="stage", bufs=1,
                                                 space="DRAM"))
        qkvg = [dstage.tile([P, L], f32r, tag=f"qkvg{n}", name=f"qkvg{n}") for n in range(NCH)]
        gstage = [dstage.tile([P, L], f32r, tag=f"gst{h}", name=f"gst{h}") for h in range(HPC)]

        # ================= Phase A: projections =================
        with ExitStack() as ctx:
            xpool = ctx.enter_context(tc.tile_pool(name="xt", bufs=1))
            xt = [None] * KC

            wpool = ctx.enter_context(tc.tile_pool(name="wc", bufs=4))
            ppool = ctx.enter_context(
                tc.tile_pool(name="proj_psum", bufs=2, space="PSUM"))
            epool = ctx.enter_context(tc.tile_pool(name="evict", bufs=2))
            tabpool = ctx.enter_context(tc.tile_pool(name="tables", bufs=1))

            cos_tab = sin_tab = None
            for n in range(NCH):
                if n == 0 or n == 4:
                    cos_tab = tabpool.tile([P, L], f32, tag="cos")
                    sin_tab = tabpool.tile([P, L], f32, tag="sin")
                    nc.sync.dma_start(cos_tab[:], cosq[:] if n == 0 else cosk[:])
                    nc.sync.dma_start(sin_tab[:], ssinq[:] if n == 0 else ssink[:])
                psum = ppool.tile([P, L], f32, tag="pp")
                for k in range(KC):
                    if xt[k] is None:
                        t = xpool.tile([P, L], f32r, tag=f"xt{k}",
                                       name=f"xtile{k}")
                        nc.sync.dma_start(t[:], xT[k * P:(k + 1) * P, :])
                        xt[k] = t
                    wc = wpool.tile([P, P], f32r, tag="wc")
                    nc.sync.dma_start(wc[:], wTb[k, n])
                    for mt in range(L // QT):
                        nc.tensor.matmul(
                            psum[:, mt * QT:(mt + 1) * QT],
                            wc[:],
                            xt[k][:, mt * QT:(mt + 1) * QT],
                            start=(k == 0),
                            stop=(k == KC - 1),
                        )
                for hf in range(2):
                    sl = slice(hf * NHALF, (hf + 1) * NHALF)
                    if n < 8:
                        raw = epool.tile([P, NHALF], f32, tag="raw")
                        nc.vector.tensor_copy(raw[:], psum[:, sl])
                        swp = epool.tile([P, NHALF], f32, tag="swp")
                        nc.sync.dma_start(swp[:64, :], raw[64:, :])
                        nc.sync.dma_start(swp[64:, :], raw[:64, :])
                        nc.vector.tensor_mul(raw[:], raw[:], cos_tab[:, sl])
                        nc.vector.tensor_mul(swp[:], swp[:], sin_tab[:, sl])
                        roped = epool.tile([P, NHALF], f32r, tag="roped")
                        nc.vector.tensor_add(roped[:], raw[:], swp[:])
                        nc.sync.dma_start(qkvg[n][:, sl], roped[:])
                    else:
                        ev = epool.tile([P, NHALF], f32r, tag="roped")
                        nc.scalar.copy(ev[:], psum[:, sl])
                        nc.sync.dma_start(qkvg[n][:, sl], ev[:])

        nc.sync.dma_start(ones[:], ones_t[:])
        nc.sync.dma_start(oneshd[:], oneshd_t[:])
        nc.sync.dma_start(ident[:], ident_t[:])
        nc.sync.dma_start(nw[:], nw_t[:])
        for r in range(4):
            nc.sync.dma_start(masks[r][:], masks_t[r])

        # ================= Phase B: attention per head =================
        with ExitStack() as ctx:
            hpool2 = ctx.enter_context(tc.tile_pool(name="headio2", bufs=2))
            hpool1 = ctx.enter_context(tc.tile_pool(name="headio1", bufs=1))
            vtp = ctx.enter_context(
                tc.tile_pool(name="vt_psum", bufs=1, space="PSUM"))
            vnpool = ctx.enter_context(tc.tile_pool(name="vnat", bufs=1))
            stp = ctx.enter_context(
                tc.tile_pool(name="st_psum", bufs=2, space="PSUM"))
            ptpool = ctx.enter_context(tc.tile_pool(name="pt", bufs=1))
            avp = ctx.enter_context(
                tc.tile_pool(name="av_psum", bufs=1, space="PSUM"))
            denp = ctx.enter_context(
                tc.tile_pool(name="den_psum", bufs=1, space="PSUM"))
            epi = ctx.enter_context(tc.tile_pool(name="epi", bufs=1))

            for h in range(HPC):
                qTt = hpool2.tile([P, L], f32r, tag="qT")
                kTt = hpool2.tile([P, L], f32r, tag="kT")
                vTt = hpool1.tile([P, L], f32r, tag="vT")
                nc.sync.dma_start(qTt[:], qkvg[h][:])
                nc.sync.dma_start(kTt[:], qkvg[4 + h][:])
                nc.sync.dma_start(vTt[:], qkvg[8 + h][:])

                vnat = []
                for c in range(CC):
                    vt_ps = vtp.tile([P, P], f32r, tag="vtp")
                    nc.tensor.transpose(
                        vt_ps[:], vTt[:, c * P:(c + 1) * P], ident[:])
                    vn = vnpool.tile([P, P], f32r, tag=f"vn{c}")
                    nc.vector.tensor_copy(vn[:], vt_ps[:])
                    vnat.append(vn)

                gTt = hpool1.tile([P, L], f32r, tag="gT")
                nc.sync.dma_start(gTt[:], qkvg[12 + h][:])
                gt = hpool1.tile([P, L], f32r, tag="gated")

                # S_T + exp + mask + AV, interleaved per kpos chunk
                av = avp.tile([P, L], f32, tag="av")
                pts = []
                for c in range(CC):
                    qs = QT * (c // 4)
                    pt = ptpool.tile([P, L - qs], f32r, tag=f"pt{c}")
                    for j in range(c // 4, L // QT):
                        ps = stp.tile([P, QT], f32, tag="st")
                        nc.tensor.matmul(
                            ps[:],
                            kTt[:, c * P:(c + 1) * P],
                            qTt[:, j * QT:(j + 1) * QT],
                            start=True, stop=True,
                        )
                        nc.scalar.activation(
                            pt[:, j * QT - qs:(j + 1) * QT - qs], ps[:], AF.Exp)
                    nc.vector.tensor_mul(
                        pt[:, 0:QT], pt[:, 0:QT], masks[c % 4][:])
                    pts.append(pt)
                    for j in range(c // 4, L // QT):
                        nc.tensor.matmul(
                            av[:, j * QT:(j + 1) * QT],
                            vnat[c][:],
                            pt[:, j * QT - qs:(j + 1) * QT - qs],
                            start=(c == 0),
                            stop=(c == 4 * j + 3),
                        )

                # evictions (DVE) + silu (ACT)
                rawh = epi.tile([P, L], f32, tag="rawh")
                nc.vector.tensor_copy(rawh[:], av[:])
                sqh = epi.tile([P, L], f32r, tag="sqh")
                nc.vector.tensor_mul(sqh[:], rawh[:], rawh[:])
                sgh = epi.tile([P, L], f32, tag="sgh")
                nc.scalar.activation(sgh[:], gTt[:], AF.Silu)
                cbh = epi.tile([P, L], f32, tag="cbh")

                # den + rms, 512-wide quarters; batch same-ACT-func ops
                dens, d2s, t2s = [], [], []
                for qq in range(L // QT):
                    den = denp.tile([P, QT], f32, tag="den")
                    for c in range(4 * qq + 4):
                        qs = QT * (c // 4)
                        nc.tensor.matmul(
                            den[:],
                            ones[:],
                            pts[c][:, qq * QT - qs:(qq + 1) * QT - qs],
                            start=(c == 0),
                            stop=(c == 4 * qq + 3),
                        )
                    dens.append(den)
                for qq in range(L // QT):
                    d2 = epi.tile([P, QT], f32, tag=f"d2_{qq}")
                    nc.scalar.activation(d2[:], dens[qq][:], AF.Square)
                    d2s.append(d2)
                for qq in range(L // QT):
                    sl = slice(qq * QT, (qq + 1) * QT)
                    s2 = stp.tile([P, QT], f32, tag="st")
                    nc.tensor.matmul(s2[:], oneshd[:], sqh[:, sl],
                                     start=True, stop=True)
                    t2 = epi.tile([P, QT], f32, tag=f"t2_{qq}")
                    nc.vector.scalar_tensor_tensor(
                        t2[:], d2s[qq][:], float(EPS), s2[:],
                        op0=OP.mult, op1=OP.add)
                    t2s.append(t2)
                for qq in range(L // QT):
                    nc.scalar.activation(t2s[qq][:], t2s[qq][:], AF.Sqrt)
                for qq in range(L // QT):
                    sl = slice(qq * QT, (qq + 1) * QT)
                    nc.vector.reciprocal(cbh[:, sl], t2s[qq][:])

                nc.vector.tensor_mul(rawh[:], rawh[:], cbh[:])
                nc.vector.scalar_tensor_tensor(
                    gt[:], rawh[:], nw[:], sgh[:],
                    op0=OP.mult, op1=OP.mult)
                nc.sync.dma_start(gstage[h][:], gt[:])

        # ================= Phase C: o_proj =================
        with ExitStack() as ctx:
            wop = ctx.enter_context(tc.tile_pool(name="wo", bufs=1))
            gpool = ctx.enter_context(tc.tile_pool(name="gres", bufs=1))
            wot, gres = [], []
            for h in range(HPC):
                t = wop.tile([P, HID], f32r, tag=f"wo{h}")
                nc.sync.dma_start(t[:], woT[h * P:(h + 1) * P, :])
                wot.append(t)
                g = gpool.tile([P, L], f32r, tag=f"gr{h}")
                nc.sync.dma_start(g[:], gstage[h][:])
                gres.append(g)
            opp = ctx.enter_context(
                tc.tile_pool(name="oproj_psum", bufs=2, space="PSUM"))
            oev = ctx.enter_context(tc.tile_pool(name="oev", bufs=3))
            for mc in range(L // P):
                ops = opp.tile([P, HID], f32, tag="op")
                for h in range(HPC):
                    for s in range(HID // QT):
                        nc.tensor.matmul(
                            ops[:, s * QT:(s + 1) * QT],
                            gres[h][:, mc * P:(mc + 1) * P],
                            wot[h][:, s * QT:(s + 1) * QT],
                            start=(h == 0),
                            stop=(h == HPC - 1),
                        )
                ot = oev.tile([P, HID], f32, tag="ot")
                nc.scalar.copy(ot[:], ops[:])
                nc.sync.dma_start(out_partial[mc * P:(mc + 1) * P, :], ot[:])

    return nc


def _host_inputs(hidden_states, wq, wk, wv, wg, wo, norm_w):
    x = np.ascontiguousarray(hidden_states.astype(np.float32))

    inv_freq = 1.0 / (ROPE_BASE ** (np.arange(0, HD, 2, dtype=np.float64) / HD))
    t = np.arange(L, dtype=np.float64)
    f = np.outer(inv_freq, t)                      # [64, L]
    cosT = np.concatenate([np.cos(f), np.cos(f)], 0)
    ssinT = np.concatenate([-np.sin(f), np.sin(f)], 0)
    cosq = np.ascontiguousarray((cosT * SCALE).astype(np.float32))
    ssinq = np.ascontiguousarray((ssinT * SCALE).astype(np.float32))
    cosk = np.ascontiguousarray(cosT.astype(np.float32))
    ssink = np.ascontiguousarray(ssinT.astype(np.float32))

    ones = np.ones((P, P), np.float32)
    oneshd = np.full((P, P), 1.0 / HD, np.float32)
    ident = np.eye(P, dtype=np.float32)
    qq = np.arange(QT)[None, :]
    kk = np.arange(P)[:, None]
    masks = np.ascontiguousarray(
        np.stack([(qq >= P * r + kk) for r in range(4)]).astype(np.float32))
    nw = np.ascontiguousarray(norm_w.astype(np.float32).reshape(P, 1))

    in_maps = []
    for c in range(NCORES):
        b, hg = c // 4, c % 4
        hs = slice(NDIM * hg, NDIM * (hg + 1))
        xTc = np.ascontiguousarray(x[b].T)
        W = np.concatenate([wq[hs], wk[hs], wv[hs], wg[hs]], 0)
        wT = np.ascontiguousarray(np.asarray(W).T.astype(np.float32))
        wTb = np.ascontiguousarray(
            wT.reshape(KC, P, NCH, P).transpose(0, 2, 1, 3))
        woTc = np.ascontiguousarray(np.asarray(wo)[:, hs].T.astype(np.float32))
        in_maps.append({
            "xT": xTc, "wTb": wTb, "woT": woTc,
            "cosq": cosq, "ssinq": ssinq, "cosk": cosk, "ssink": ssink,
            "ones_t": ones, "oneshd_t": oneshd, "ident_t": ident,
            "masks_t": masks, "nw_t": nw,
        })
    return in_maps


_NC_CACHE = {}


def _get_nc():
    if "nc" not in _NC_CACHE:
        import concourse.bacc as bacc
        import concourse.mybir as mybir
        import concourse.tile as tile
        nc = bacc.Bacc("TRN2", target_bir_lowering=False, debug=False)
        _build(nc, mybir, tile)
        nc.compile()
        _NC_CACHE["nc"] = nc
    return _NC_CACHE["nc"]


def kernel(hidden_states, wq, wk, wv, wg, wo, norm_w, _trace=False):
    from concourse.bass_utils import run_bass_kernel_spmd

    nc = _get_nc()
    in_maps = _host_inputs(np.asarray(hidden_states), np.asarray(wq),
                           np.asarray(wk), np.asarray(wv), np.asarray(wg),
                           np.asarray(wo), np.asarray(norm_w))
    res = run_bass_kernel_spmd(nc, in_maps, list(range(NCORES)), trace=_trace)
    out = np.zeros((B, L, HID), np.float32)
    for c in range(NCORES):
        out[c // 4] += res.results[c]["out_partial"]
    if _trace:
        kernel._last_results = res
    return out



# revision 2
# speedup vs baseline: 18.9596x; 18.9596x over previous
"""Gated causal attention (B=2, L=2048, HID=2048, NH=16, HD=128) on 8 trn2 cores.

Sharding: data-parallel over batch (cores 0-3 batch 0, cores 4-7 batch 1) x
tensor-parallel over heads (4 heads per core within its batch).

The axon tunnel (~40MB/s up, ~30MB/s down) dominates wall time, so the
pipeline is built around minimizing wire bytes (~56MB up, 16MB down):
  1. host: cast inputs to fp16 and lay them out so every byte crosses the
     wire exactly once (x sharded by L-quarters within its batch group,
     weights sharded in halves across the two batch groups).
  2. gather jit (stock XLA on-device): grouped all_gathers replicate x
     within batch groups and weight slices across batch groups, then
     transpose/re-block into the layouts the bass kernel consumes; also
     materializes RoPE tables / masks / consts / output zero-buffers
     on-device as executable constants.
  3. bass jit (this file's _build): per-core fp16 projections + RoPE +
     causal attention + RMSNorm + silu gating + o_proj partial, f32 out.
  4. psum jit: on-device grouped all-reduce of the 4 partials per batch,
     cast fp16; only shards 0 and 4 (one per batch) are fetched.
All three jits are cached in module state across kernel() calls.
"""

import numpy as np

B, L, HID, NH, HD = 2, 2048, 2048, 16, 128
EPS = 1e-5
SCALE = HD ** -0.5
ROPE_BASE = 10000.0
NCORES = 8
HPC = 4            # heads per core
NDIM = HPC * HD    # 512 projection dims per core
P = 128
KC = HID // P      # 16 k-chunks
CC = L // P        # 16 kpos chunks
QT = 512           # q tile (moving max for f32 psum)
NHALF = L // 2     # AV/den psum half width
NCH = (4 * NDIM) // P  # 16 fused projection n-chunks (q|k|v|g)


def _rope_tables():
    inv_freq = 1.0 / (ROPE_BASE ** (np.arange(0, HD, 2, dtype=np.float64) / HD))
    t = np.arange(L, dtype=np.float64)
    f = np.outer(inv_freq, t)                      # [64, L]
    cosT = np.concatenate([np.cos(f), np.cos(f)], 0)
    ssinT = np.concatenate([-np.sin(f), np.sin(f)], 0)
    return (
        np.ascontiguousarray((cosT * SCALE).astype(np.float32)),
        np.ascontiguousarray((ssinT * SCALE).astype(np.float32)),
        np.ascontiguousarray(cosT.astype(np.float32)),
        np.ascontiguousarray(ssinT.astype(np.float32)),
    )


_COSQ, _SSINQ, _COSK, _SSINK = _rope_tables()
_MASKS = np.ascontiguousarray(
    np.stack([(np.arange(QT)[None, :] >= P * r + np.arange(P)[:, None])
              for r in range(4)]).astype(np.float16))


def _build(nc, mybir, tile):
    from contextlib import ExitStack

    f16 = mybir.dt.float16
    bf = mybir.dt.bfloat16
    f32 = mybir.dt.float32
    AF = mybir.ActivationFunctionType
    OP = mybir.AluOpType

    xT = nc.dram_tensor("xT", [HID, L], f16, kind="ExternalInput")
    # wT blocked: [k-chunk, n-chunk, 128, 128]; n order = q|k|v|g, each 512
    wTb = nc.dram_tensor("wTb", [KC, NCH, P, P], f16, kind="ExternalInput")
    woT = nc.dram_tensor("woT", [NDIM, HID], f16, kind="ExternalInput")
    cosq = nc.dram_tensor("cosq", [P, L], f32, kind="ExternalInput")
    ssinq = nc.dram_tensor("ssinq", [P, L], f32, kind="ExternalInput")
    cosk = nc.dram_tensor("cosk", [P, L], f32, kind="ExternalInput")
    ssink = nc.dram_tensor("ssink", [P, L], f32, kind="ExternalInput")
    ones_t = nc.dram_tensor("ones_t", [P, P], f16, kind="ExternalInput")
    oneshd_t = nc.dram_tensor("oneshd_t", [P, P], bf, kind="ExternalInput")
    ident_t = nc.dram_tensor("ident_t", [P, P], f16, kind="ExternalInput")
    masks_t = nc.dram_tensor("masks_t", [4, P, QT], f16, kind="ExternalInput")
    nw_t = nc.dram_tensor("nw_t", [P, 1], f32, kind="ExternalInput")
    out_partial = nc.dram_tensor("out_partial", [L, HID], f32,
                                 kind="ExternalOutput")

    with tile.TileContext(nc) as tc, ExitStack() as octx:
        const = octx.enter_context(tc.tile_pool(name="const", bufs=1))
        ones = const.tile([P, P], f16, tag="ones")
        oneshd = const.tile([P, P], bf, tag="oneshd")
        ident = const.tile([P, P], f16, tag="ident")
        nw = const.tile([P, 1], f32, tag="nw")
        masks = [const.tile([P, QT], f16, tag=f"mask{r}", name=f"mask{r}") for r in range(4)]

        # DRAM staging pools (tracked by Tile)
        dstage = octx.enter_context(tc.tile_pool(name="stage", bufs=1,
                                                 space="DRAM"))
        qkvg = [dstage.tile([P, L], f16, tag=f"qkvg{n}", name=f"qkvg{n}") for n in range(NCH)]
        gstage = [dstage.tile([P, L], f16, tag=f"gst{h}", name=f"gst{h}") for h in range(HPC)]

        # ================= Phase A: projections =================
        with ExitStack() as ctx:
            xpool = ctx.enter_context(tc.tile_pool(name="xt", bufs=1))
            xt = [None] * KC

            wpool = ctx.enter_context(tc.tile_pool(name="wc", bufs=4))
            ppool = ctx.enter_context(
                tc.tile_pool(name="proj_psum", bufs=2, space="PSUM"))
            epool = ctx.enter_context(tc.tile_pool(name="evict", bufs=2))
            tabpool = ctx.enter_context(tc.tile_pool(name="tables", bufs=1))

            cos_tab = sin_tab = None
            for n in range(NCH):
                if n == 0 or n == 4:
                    cos_tab = tabpool.tile([P, L], f32, tag="cos")
                    sin_tab = tabpool.tile([P, L], f32, tag="sin")
                    nc.sync.dma_start(cos_tab[:], cosq[:] if n == 0 else cosk[:])
                    nc.sync.dma_start(sin_tab[:], ssinq[:] if n == 0 else ssink[:])
                psum = ppool.tile([P, L], f32, tag="pp")
                for k in range(KC):
                    if xt[k] is None:
                        t = xpool.tile([P, L], f16, tag=f"xt{k}",
                                       name=f"xtile{k}")
                        nc.sync.dma_start(t[:], xT[k * P:(k + 1) * P, :])
                        xt[k] = t
                    wc = wpool.tile([P, P], f16, tag="wc")
                    nc.sync.dma_start(wc[:], wTb[k, n])
                    for mt in range(L // QT):
                        nc.tensor.matmul(
                            psum[:, mt * QT:(mt + 1) * QT],
                            wc[:],
                            xt[k][:, mt * QT:(mt + 1) * QT],
                            start=(k == 0),
                            stop=(k == KC - 1),
                        )
                for hf in range(2):
                    sl = slice(hf * NHALF, (hf + 1) * NHALF)
                    if n < 8:
                        raw = epool.tile([P, NHALF], f32, tag="raw")
                        nc.vector.tensor_copy(raw[:], psum[:, sl])
                        swp = epool.tile([P, NHALF], f32, tag="swp")
                        nc.sync.dma_start(swp[:64, :], raw[64:, :])
                        nc.sync.dma_start(swp[64:, :], raw[:64, :])
                        nc.vector.tensor_mul(raw[:], raw[:], cos_tab[:, sl])
                        nc.vector.tensor_mul(swp[:], swp[:], sin_tab[:, sl])
                        roped = epool.tile([P, NHALF], f16, tag="roped")
                        nc.vector.tensor_add(roped[:], raw[:], swp[:])
                        nc.sync.dma_start(qkvg[n][:, sl], roped[:])
                    else:
                        ev = epool.tile([P, NHALF], f16, tag="roped")
                        nc.scalar.copy(ev[:], psum[:, sl])
                        nc.sync.dma_start(qkvg[n][:, sl], ev[:])

        nc.sync.dma_start(ones[:], ones_t[:])
        nc.sync.dma_start(oneshd[:], oneshd_t[:])
        nc.sync.dma_start(ident[:], ident_t[:])
        nc.sync.dma_start(nw[:], nw_t[:])
        for r in range(4):
            nc.sync.dma_start(masks[r][:], masks_t[r])

        # ================= Phase B: attention per head =================
        with ExitStack() as ctx:
            hpool2 = ctx.enter_context(tc.tile_pool(name="headio2", bufs=2))
            hpool1 = ctx.enter_context(tc.tile_pool(name="headio1", bufs=1))
            vtp = ctx.enter_context(
                tc.tile_pool(name="vt_psum", bufs=1, space="PSUM"))
            vnpool = ctx.enter_context(tc.tile_pool(name="vnat", bufs=1))
            stp = ctx.enter_context(
                tc.tile_pool(name="st_psum", bufs=2, space="PSUM"))
            ptpool = ctx.enter_context(tc.tile_pool(name="pt", bufs=1))
            avp = ctx.enter_context(
                tc.tile_pool(name="av_psum", bufs=1, space="PSUM"))
            denp = ctx.enter_context(
                tc.tile_pool(name="den_psum", bufs=1, space="PSUM"))
            epi = ctx.enter_context(tc.tile_pool(name="epi", bufs=1))

            for h in range(HPC):
                qTt = hpool2.tile([P, L], f16, tag="qT")
                kTt = hpool2.tile([P, L], f16, tag="kT")
                vTt = hpool1.tile([P, L], f16, tag="vT")
                nc.sync.dma_start(qTt[:], qkvg[h][:])
                nc.sync.dma_start(kTt[:], qkvg[4 + h][:])
                nc.sync.dma_start(vTt[:], qkvg[8 + h][:])

                vnat = []
                for c in range(CC):
                    vt_ps = vtp.tile([P, P], f16, tag="vtp")
                    nc.tensor.transpose(
                        vt_ps[:], vTt[:, c * P:(c + 1) * P], ident[:])
                    vn = vnpool.tile([P, P], f16, tag=f"vn{c}")
                    nc.vector.tensor_copy(vn[:], vt_ps[:])
                    vnat.append(vn)

                gTt = hpool1.tile([P, L], f16, tag="gT")
                nc.sync.dma_start(gTt[:], qkvg[12 + h][:])
                gt = hpool1.tile([P, L], f16, tag="gated")

                # S_T + exp + mask + AV, interleaved per kpos chunk
                av = avp.tile([P, L], f32, tag="av")
                pts = []
                for c in range(CC):
                    qs = QT * (c // 4)
                    pt = ptpool.tile([P, L - qs], f16, tag=f"pt{c}")
                    for j in range(c // 4, L // QT):
                        ps = stp.tile([P, QT], f32, tag="st")
                        nc.tensor.matmul(
                            ps[:],
                            kTt[:, c * P:(c + 1) * P],
                            qTt[:, j * QT:(j + 1) * QT],
                            start=True, stop=True,
                        )
                        nc.scalar.activation(
                            pt[:, j * QT - qs:(j + 1) * QT - qs], ps[:], AF.Exp)
                    nc.vector.tensor_mul(
                        pt[:, 0:QT], pt[:, 0:QT], masks[c % 4][:])
                    pts.append(pt)
                    for j in range(c // 4, L // QT):
                        nc.tensor.matmul(
                            av[:, j * QT:(j + 1) * QT],
                            vnat[c][:],
                            pt[:, j * QT - qs:(j + 1) * QT - qs],
                            start=(c == 0),
                            stop=(c == 4 * j + 3),
                        )

                # evictions (DVE) + silu (ACT)
                rawh = epi.tile([P, L], f32, tag="rawh")
                nc.vector.tensor_copy(rawh[:], av[:])
                sqh = epi.tile([P, L], bf, tag="sqh")
                nc.vector.tensor_mul(sqh[:], rawh[:], rawh[:])
                sgh = epi.tile([P, L], f32, tag="sgh")
                nc.scalar.activation(sgh[:], gTt[:], AF.Silu)
                cbh = epi.tile([P, L], f32, tag="cbh")

                # den + rms, 512-wide quarters; batch same-ACT-func ops
                dens, d2s, t2s = [], [], []
                for qq in range(L // QT):
                    den = denp.tile([P, QT], f32, tag="den")
                    for c in range(4 * qq + 4):
                        qs = QT * (c // 4)
                        nc.tensor.matmul(
                            den[:],
                            ones[:],
                            pts[c][:, qq * QT - qs:(qq + 1) * QT - qs],
                            start=(c == 0),
                            stop=(c == 4 * qq + 3),
                        )
                    dens.append(den)
                for qq in range(L // QT):
                    d2 = epi.tile([P, QT], f32, tag=f"d2_{qq}")
                    nc.scalar.activation(d2[:], dens[qq][:], AF.Square)
                    d2s.append(d2)
                for qq in range(L // QT):
                    sl = slice(qq * QT, (qq + 1) * QT)
                    s2 = stp.tile([P, QT], f32, tag="st")
                    nc.tensor.matmul(s2[:], oneshd[:], sqh[:, sl],
                                     start=True, stop=True)
                    t2 = epi.tile([P, QT], f32, tag=f"t2_{qq}")
                    nc.vector.scalar_tensor_tensor(
                        t2[:], d2s[qq][:], float(EPS), s2[:],
                        op0=OP.mult, op1=OP.add)
                    t2s.append(t2)
                for qq in range(L // QT):
                    nc.scalar.activation(t2s[qq][:], t2s[qq][:], AF.Sqrt)
                for qq in range(L // QT):
                    sl = slice(qq * QT, (qq + 1) * QT)
                    nc.vector.reciprocal(cbh[:, sl], t2s[qq][:])

                nc.vector.tensor_mul(rawh[:], rawh[:], cbh[:])
                nc.vector.scalar_tensor_tensor(
                    gt[:], rawh[:], nw[:], sgh[:],
                    op0=OP.mult, op1=OP.mult)
                nc.sync.dma_start(gstage[h][:], gt[:])

        # ================= Phase C: o_proj =================
        with ExitStack() as ctx:
            wop = ctx.enter_context(tc.tile_pool(name="wo", bufs=1))
            gpool = ctx.enter_context(tc.tile_pool(name="gres", bufs=1))
            wot, gres = [], []
            for h in range(HPC):
                t = wop.tile([P, HID], f16, tag=f"wo{h}")
                nc.sync.dma_start(t[:], woT[h * P:(h + 1) * P, :])
                wot.append(t)
                g = gpool.tile([P, L], f16, tag=f"gr{h}")
                nc.sync.dma_start(g[:], gstage[h][:])
                gres.append(g)
            opp = ctx.enter_context(
                tc.tile_pool(name="oproj_psum", bufs=2, space="PSUM"))
            oev = ctx.enter_context(tc.tile_pool(name="oev", bufs=3))
            for mc in range(L // P):
                ops = opp.tile([P, HID], f32, tag="op")
                for h in range(HPC):
                    for s in range(HID // QT):
                        nc.tensor.matmul(
                            ops[:, s * QT:(s + 1) * QT],
                            gres[h][:, mc * P:(mc + 1) * P],
                            wot[h][:, s * QT:(s + 1) * QT],
                            start=(h == 0),
                            stop=(h == HPC - 1),
                        )
                ot = oev.tile([P, HID], f32, tag="ot")
                nc.scalar.copy(ot[:], ops[:])
                nc.sync.dma_start(out_partial[mc * P:(mc + 1) * P, :], ot[:])

    return nc


_STATE = {}


def _get_state():
    if _STATE:
        return _STATE
    import jax
    import jax.numpy as jnp
    from jax.sharding import Mesh, PartitionSpec, NamedSharding
    from jax.experimental.shard_map import shard_map
    import concourse.bacc as bacc
    import concourse.mybir as mybir
    import concourse.tile as tile
    from concourse.bass2jax import (
        _bass_exec_p, partition_id_tensor, install_neuronx_cc_hook)

    install_neuronx_cc_hook()
    nc = bacc.Bacc("TRN2", target_bir_lowering=False, debug=False)
    _build(nc, mybir, tile)
    nc.compile()

    partition_name = (nc.partition_id_tensor.name
                      if nc.partition_id_tensor is not None else None)
    in_names, out_names, out_avals = [], [], []
    for alloc in nc.m.functions[0].allocations:
        if not isinstance(alloc, mybir.MemoryLocationSet):
            continue
        name = alloc.memorylocations[0].name
        if alloc.kind == "ExternalInput":
            if name != partition_name:
                in_names.append(name)
        elif alloc.kind == "ExternalOutput":
            shape = tuple(alloc.tensor_shape)
            dtype = mybir.dt.np(alloc.dtype)
            out_names.append(name)
            out_avals.append(jax.core.ShapedArray(shape, dtype))
    n_params = len(in_names)
    n_outs = len(out_names)
    all_in_names = list(in_names) + list(out_names)
    if partition_name is not None:
        all_in_names.append(partition_name)

    devs = jax.devices()[:NCORES]
    mesh = Mesh(np.asarray(devs), ("core",))
    Pc = PartitionSpec("core")
    shc = NamedSharding(mesh, Pc)

    def bass_body(*args):
        operands = list(args)
        if partition_name is not None:
            operands.append(partition_id_tensor())
        outs = _bass_exec_p.bind(
            *operands,
            out_avals=tuple(out_avals),
            in_names=tuple(all_in_names),
            out_names=tuple(out_names),
            lowering_input_output_aliases=(),
            sim_require_finite=True,
            sim_require_nnan=True,
            nc=nc,
        )
        return tuple(outs)

    bass_jit = jax.jit(
        shard_map(bass_body, mesh=mesh,
                  in_specs=(Pc,) * (n_params + n_outs),
                  out_specs=(Pc,) * n_outs, check_rep=False),
        donate_argnums=tuple(range(n_params, n_params + n_outs)),
        keep_unused=True,
    )

    cosq_c, ssinq_c, cosk_c, ssink_c = _COSQ, _SSINQ, _COSK, _SSINK
    masks_c = _MASKS

    def gather_body(x_sh, w_sh, wo_sh, nw_sh):
        xb = jax.lax.all_gather(
            x_sh, "core", axis_index_groups=[[0, 1, 2, 3], [4, 5, 6, 7]],
            tiled=True)                                   # [L, HID] f16
        xTg = xb.T                                        # [HID, L]
        wfull = jax.lax.all_gather(
            w_sh, "core", axis_index_groups=[[0, 4], [1, 5], [2, 6], [3, 7]],
            tiled=True)                                   # [NCH*P, HID] f16
        wTbg = wfull.T.reshape(KC, P, NCH, P).transpose(0, 2, 1, 3)
        woTg = jax.lax.all_gather(
            wo_sh, "core", axis_index_groups=[[0, 4], [1, 5], [2, 6], [3, 7]],
            tiled=True)                                   # [NDIM, HID] f16
        ones = jnp.ones((P, P), jnp.float16)
        oneshd = jnp.full((P, P), 1.0 / HD, jnp.bfloat16)
        ident = jnp.eye(P, dtype=jnp.float16)
        zeros = jnp.zeros((L, HID), jnp.float32)
        return (xTg, wTbg, woTg, jnp.asarray(cosq_c), jnp.asarray(ssinq_c),
                jnp.asarray(cosk_c), jnp.asarray(ssink_c),
                jnp.asarray(masks_c), ones, oneshd, ident, nw_sh, zeros)

    gather_jit = jax.jit(
        shard_map(gather_body, mesh=mesh, in_specs=(Pc,) * 4,
                  out_specs=(Pc,) * 13, check_rep=False))

    def psum_body(p):
        s = jax.lax.psum(p, "core",
                         axis_index_groups=[[0, 1, 2, 3], [4, 5, 6, 7]])
        return s.astype(jnp.float16)

    psum_jit = jax.jit(
        shard_map(psum_body, mesh=mesh, in_specs=Pc, out_specs=Pc,
                  check_rep=False))

    _STATE.update(
        jax=jax, mesh=mesh, shc=shc, in_names=in_names, out_names=out_names,
        gather_jit=gather_jit, bass_jit=bass_jit, psum_jit=psum_jit)
    return _STATE


_GNAMES = ["xT", "wTb", "woT", "cosq", "ssinq", "cosk", "ssink",
           "masks_t", "ones_t", "oneshd_t", "ident_t", "nw_t", "__zeros__"]


def kernel(hidden_states, wq, wk, wv, wg, wo, norm_w, _trace=False):
    st = _get_state()
    jax = st["jax"]

    # host prep: fp16 casts laid out so each byte crosses the wire once
    x_st = np.asarray(hidden_states, np.float32).reshape(
        NCORES * QT, HID).astype(np.float16)
    xd = jax.device_put(x_st, st["shc"])

    wqh = np.asarray(wq, np.float32).astype(np.float16)
    wkh = np.asarray(wk, np.float32).astype(np.float16)
    wvh = np.asarray(wv, np.float32).astype(np.float16)
    wgh = np.asarray(wg, np.float32).astype(np.float16)
    w_st = np.empty((8 * 1024, HID), np.float16)
    for g in range(4):
        hs = slice(NDIM * g, NDIM * (g + 1))
        w_st[1024 * g:1024 * g + 512] = wqh[hs]
        w_st[1024 * g + 512:1024 * (g + 1)] = wkh[hs]
        w_st[4096 + 1024 * g:4096 + 1024 * g + 512] = wvh[hs]
        w_st[4096 + 1024 * g + 512:4096 + 1024 * (g + 1)] = wgh[hs]
    wd = jax.device_put(w_st, st["shc"])

    woT16 = np.ascontiguousarray(np.asarray(wo, np.float32)
                                 .astype(np.float16).T)
    wo_st = np.ascontiguousarray(
        woT16.reshape(4, 2, 256, HID).transpose(1, 0, 2, 3).reshape(2048, HID))
    wod = jax.device_put(wo_st, st["shc"])

    nw_g = np.tile(np.asarray(norm_w, np.float32).reshape(P, 1), (NCORES, 1))
    nwd = jax.device_put(nw_g, st["shc"])

    g_outs = st["gather_jit"](xd, wd, wod, nwd)
    gmap = dict(zip(_GNAMES, g_outs))
    bass_args = [gmap[n] for n in st["in_names"]] + [gmap["__zeros__"]]
    (partial,) = st["bass_jit"](*bass_args)
    summed = st["psum_jit"](partial)

    out = np.empty((B, L, HID), np.float32)
    for shard in summed.addressable_shards:
        start = shard.index[0].start or 0
        if start == 0:
            out[0] = np.asarray(shard.data, np.float32)
        elif start == 4 * L:
            out[1] = np.asarray(shard.data, np.float32)
    return out


# revision 3
# speedup vs baseline: 20.4148x; 1.0767x over previous
"""Gated causal attention (B=2, L=2048, HID=2048, NH=16, HD=128) on 8 trn2 cores.

Sharding: data-parallel over batch (cores 0-3 batch 0, cores 4-7 batch 1) x
tensor-parallel over heads (4 heads per core within its batch).

The axon tunnel (~40MB/s up, ~30MB/s down) dominates wall time, so the
pipeline is built around minimizing wire bytes (~56MB up, 16MB down):
  1. host: cast inputs to fp16 and lay them out so every byte crosses the
     wire exactly once (x sharded by L-quarters within its batch group,
     weights sharded in halves across the two batch groups).
  2. gather jit (stock XLA on-device): grouped all_gathers replicate x
     within batch groups and weight slices across batch groups, then
     transpose/re-block into the layouts the bass kernel consumes; also
     materializes RoPE tables / masks / consts / output zero-buffers
     on-device as executable constants.
  3. bass jit (this file's _build): per-core fp16 projections + RoPE +
     causal attention + RMSNorm + silu gating + o_proj partial, f32 out.
  4. psum jit: on-device grouped all-reduce of the 4 partials per batch,
     cast fp16; only shards 0 and 4 (one per batch) are fetched.
All three jits are cached in module state across kernel() calls.
"""

import numpy as np

B, L, HID, NH, HD = 2, 2048, 2048, 16, 128
EPS = 1e-5
SCALE = HD ** -0.5
ROPE_BASE = 10000.0
NCORES = 8
HPC = 4            # heads per core
NDIM = HPC * HD    # 512 projection dims per core
P = 128
KC = HID // P      # 16 k-chunks
CC = L // P        # 16 kpos chunks
QT = 512           # q tile (moving max for f32 psum)
NHALF = L // 2     # AV/den psum half width
NCH = (4 * NDIM) // P  # 16 fused projection n-chunks (q|k|v|g)


def _rope_tables():
    inv_freq = 1.0 / (ROPE_BASE ** (np.arange(0, HD, 2, dtype=np.float64) / HD))
    t = np.arange(L, dtype=np.float64)
    f = np.outer(inv_freq, t)                      # [64, L]
    cosT = np.concatenate([np.cos(f), np.cos(f)], 0)
    ssinT = np.concatenate([-np.sin(f), np.sin(f)], 0)
    return (
        np.ascontiguousarray((cosT * SCALE).astype(np.float32)),
        np.ascontiguousarray((ssinT * SCALE).astype(np.float32)),
        np.ascontiguousarray(cosT.astype(np.float32)),
        np.ascontiguousarray(ssinT.astype(np.float32)),
    )


_COSQ, _SSINQ, _COSK, _SSINK = _rope_tables()
_MASKS = np.ascontiguousarray(
    np.stack([(np.arange(QT)[None, :] >= P * r + np.arange(P)[:, None])
              for r in range(4)]).astype(np.float16))


def _build(nc, mybir, tile):
    from contextlib import ExitStack

    f16 = mybir.dt.float16
    bf = mybir.dt.bfloat16
    f32 = mybir.dt.float32
    AF = mybir.ActivationFunctionType
    OP = mybir.AluOpType

    xT = nc.dram_tensor("xT", [HID, L], f16, kind="ExternalInput")
    # wT blocked: [k-chunk, n-chunk, 128, 128]; n order = q|k|v|g, each 512
    wTb = nc.dram_tensor("wTb", [KC, NCH, P, P], f16, kind="ExternalInput")
    woT = nc.dram_tensor("woT", [NDIM, HID], f16, kind="ExternalInput")
    cosq = nc.dram_tensor("cosq", [P, L], f32, kind="ExternalInput")
    ssinq = nc.dram_tensor("ssinq", [P, L], f32, kind="ExternalInput")
    cosk = nc.dram_tensor("cosk", [P, L], f32, kind="ExternalInput")
    ssink = nc.dram_tensor("ssink", [P, L], f32, kind="ExternalInput")
    ones_t = nc.dram_tensor("ones_t", [P, P], f16, kind="ExternalInput")
    oneshd_t = nc.dram_tensor("oneshd_t", [P, P], bf, kind="ExternalInput")
    ident_t = nc.dram_tensor("ident_t", [P, P], f16, kind="ExternalInput")
    masks_t = nc.dram_tensor("masks_t", [4, P, QT], f16, kind="ExternalInput")
    nw_t = nc.dram_tensor("nw_t", [P, 1], f32, kind="ExternalInput")
    out_partial = nc.dram_tensor("out_partial", [L, HID], f32,
                                 kind="ExternalOutput")

    with tile.TileContext(nc) as tc, ExitStack() as octx:
        const = octx.enter_context(tc.tile_pool(name="const", bufs=1))
        ones = const.tile([P, P], f16, tag="ones")
        oneshd = const.tile([P, P], bf, tag="oneshd")
        ident = const.tile([P, P], f16, tag="ident")
        nw = const.tile([P, 1], f32, tag="nw")
        masks = [const.tile([P, QT], f16, tag=f"mask{r}", name=f"mask{r}") for r in range(4)]

        # DRAM staging pools (tracked by Tile)
        dstage = octx.enter_context(tc.tile_pool(name="stage", bufs=1,
                                                 space="DRAM"))
        qkvg = [dstage.tile([P, L], f16, tag=f"qkvg{n}", name=f"qkvg{n}") for n in range(NCH)]
        gstage = [dstage.tile([P, L], f16, tag=f"gst{h}", name=f"gst{h}") for h in range(HPC)]

        # ================= Phase A: projections =================
        with ExitStack() as ctx:
            xpool = ctx.enter_context(tc.tile_pool(name="xt", bufs=1))
            xt = [None] * KC

            wpool = ctx.enter_context(tc.tile_pool(name="wc", bufs=4))
            ppool = ctx.enter_context(
                tc.tile_pool(name="proj_psum", bufs=2, space="PSUM"))
            epool = ctx.enter_context(tc.tile_pool(name="evict", bufs=2))
            tabpool = ctx.enter_context(tc.tile_pool(name="tables", bufs=1))

            cos_tab = sin_tab = None
            for n in range(NCH):
                if n == 0 or n == 4:
                    cos_tab = tabpool.tile([P, L], f32, tag="cos")
                    sin_tab = tabpool.tile([P, L], f32, tag="sin")
                    nc.sync.dma_start(cos_tab[:], cosq[:] if n == 0 else cosk[:])
                    nc.sync.dma_start(sin_tab[:], ssinq[:] if n == 0 else ssink[:])
                psum = ppool.tile([P, L], f32, tag="pp")
                for k in range(KC):
                    if xt[k] is None:
                        t = xpool.tile([P, L], f16, tag=f"xt{k}",
                                       name=f"xtile{k}")
                        nc.sync.dma_start(t[:], xT[k * P:(k + 1) * P, :])
                        xt[k] = t
                    wc = wpool.tile([P, P], f16, tag="wc")
                    nc.sync.dma_start(wc[:], wTb[k, n])
                    for mt in range(L // QT):
                        nc.tensor.matmul(
                            psum[:, mt * QT:(mt + 1) * QT],
                            wc[:],
                            xt[k][:, mt * QT:(mt + 1) * QT],
                            start=(k == 0),
                            stop=(k == KC - 1),
                        )
                for hf in range(2):
                    sl = slice(hf * NHALF, (hf + 1) * NHALF)
                    if n < 8:
                        raw = epool.tile([P, NHALF], f32, tag="raw")
                        nc.vector.tensor_copy(raw[:], psum[:, sl])
                        swp = epool.tile([P, NHALF], f32, tag="swp")
                        nc.sync.dma_start(swp[:64, :], raw[64:, :])
                        nc.sync.dma_start(swp[64:, :], raw[:64, :])
                        nc.vector.tensor_mul(raw[:], raw[:], cos_tab[:, sl])
                        nc.vector.tensor_mul(swp[:], swp[:], sin_tab[:, sl])
                        roped = epool.tile([P, NHALF], f16, tag="roped")
                        nc.vector.tensor_add(roped[:], raw[:], swp[:])
                        nc.sync.dma_start(qkvg[n][:, sl], roped[:])
                    else:
                        ev = epool.tile([P, NHALF], f16, tag="roped")
                        nc.scalar.copy(ev[:], psum[:, sl])
                        nc.sync.dma_start(qkvg[n][:, sl], ev[:])

        nc.sync.dma_start(ones[:], ones_t[:])
        nc.sync.dma_start(oneshd[:], oneshd_t[:])
        nc.sync.dma_start(ident[:], ident_t[:])
        nc.sync.dma_start(nw[:], nw_t[:])
        for r in range(4):
            nc.sync.dma_start(masks[r][:], masks_t[r])

        # ================= Phase B: attention per head =================
        with ExitStack() as ctx:
            hpool2 = ctx.enter_context(tc.tile_pool(name="headio2", bufs=2))
            hpool1 = ctx.enter_context(tc.tile_pool(name="headio1", bufs=1))
            vtp = ctx.enter_context(
                tc.tile_pool(name="vt_psum", bufs=1, space="PSUM"))
            vnpool = ctx.enter_context(tc.tile_pool(name="vnat", bufs=1))
            stp = ctx.enter_context(
                tc.tile_pool(name="st_psum", bufs=2, space="PSUM"))
            ptpool = ctx.enter_context(tc.tile_pool(name="pt", bufs=1))
            avp = ctx.enter_context(
                tc.tile_pool(name="av_psum", bufs=1, space="PSUM"))
            denp = ctx.enter_context(
                tc.tile_pool(name="den_psum", bufs=1, space="PSUM"))
            epi = ctx.enter_context(tc.tile_pool(name="epi", bufs=1))

            for h in range(HPC):
                qTt = hpool2.tile([P, L], f16, tag="qT")
                kTt = hpool2.tile([P, L], f16, tag="kT")
                vTt = hpool1.tile([P, L], f16, tag="vT")
                nc.sync.dma_start(qTt[:], qkvg[h][:])
                nc.sync.dma_start(kTt[:], qkvg[4 + h][:])
                nc.sync.dma_start(vTt[:], qkvg[8 + h][:])

                vnat = []
                for c in range(CC):
                    vt_ps = vtp.tile([P, P], f16, tag="vtp")
                    nc.tensor.transpose(
                        vt_ps[:], vTt[:, c * P:(c + 1) * P], ident[:])
                    vn = vnpool.tile([P, P], f16, tag=f"vn{c}")
                    nc.vector.tensor_copy(vn[:], vt_ps[:])
                    vnat.append(vn)

                gTt = hpool1.tile([P, L], f16, tag="gT")
                nc.sync.dma_start(gTt[:], qkvg[12 + h][:])
                gt = hpool1.tile([P, L], f16, tag="gated")

                # S_T + exp + mask + AV, interleaved per kpos chunk
                av = avp.tile([P, L], f32, tag="av")
                pts = []
                for c in range(CC):
                    qs = QT * (c // 4)
                    pt = ptpool.tile([P, L - qs], f16, tag=f"pt{c}")
                    for j in range(c // 4, L // QT):
                        ps = stp.tile([P, QT], f32, tag="st")
                        nc.tensor.matmul(
                            ps[:],
                            kTt[:, c * P:(c + 1) * P],
                            qTt[:, j * QT:(j + 1) * QT],
                            start=True, stop=True,
                        )
                        nc.scalar.activation(
                            pt[:, j * QT - qs:(j + 1) * QT - qs], ps[:], AF.Exp)
                    nc.vector.tensor_mul(
                        pt[:, 0:QT], pt[:, 0:QT], masks[c % 4][:])
                    pts.append(pt)
                    for j in range(c // 4, L // QT):
                        nc.tensor.matmul(
                            av[:, j * QT:(j + 1) * QT],
                            vnat[c][:],
                            pt[:, j * QT - qs:(j + 1) * QT - qs],
                            start=(c == 0),
                            stop=(c == 4 * j + 3),
                        )

                # evictions (DVE) + silu (ACT)
                rawh = epi.tile([P, L], f32, tag="rawh")
                nc.vector.tensor_copy(rawh[:], av[:])
                sqh = epi.tile([P, L], bf, tag="sqh")
                nc.vector.tensor_mul(sqh[:], rawh[:], rawh[:])
                sgh = epi.tile([P, L], f32, tag="sgh")
                nc.scalar.activation(sgh[:], gTt[:], AF.Silu)
                cbh = epi.tile([P, L], f32, tag="cbh")

                # den + rms, 512-wide quarters; batch same-ACT-func ops
                dens, d2s, t2s = [], [], []
                for qq in range(L // QT):
                    den = denp.tile([P, QT], f32, tag="den")
                    for c in range(4 * qq + 4):
                        qs = QT * (c // 4)
                        nc.tensor.matmul(
                            den[:],
                            ones[:],
                            pts[c][:, qq * QT - qs:(qq + 1) * QT - qs],
                            start=(c == 0),
                            stop=(c == 4 * qq + 3),
                        )
                    dens.append(den)
                for qq in range(L // QT):
                    d2 = epi.tile([P, QT], f32, tag=f"d2_{qq}")
                    nc.scalar.activation(d2[:], dens[qq][:], AF.Square)
                    d2s.append(d2)
                for qq in range(L // QT):
                    sl = slice(qq * QT, (qq + 1) * QT)
                    s2 = stp.tile([P, QT], f32, tag="st")
                    nc.tensor.matmul(s2[:], oneshd[:], sqh[:, sl],
                                     start=True, stop=True)
                    t2 = epi.tile([P, QT], f32, tag=f"t2_{qq}")
                    nc.vector.scalar_tensor_tensor(
                        t2[:], d2s[qq][:], float(EPS), s2[:],
                        op0=OP.mult, op1=OP.add)
                    t2s.append(t2)
                for qq in range(L // QT):
                    nc.scalar.activation(t2s[qq][:], t2s[qq][:], AF.Sqrt)
                for qq in range(L // QT):
                    sl = slice(qq * QT, (qq + 1) * QT)
                    nc.vector.reciprocal(cbh[:, sl], t2s[qq][:])

                nc.vector.tensor_mul(rawh[:], rawh[:], cbh[:])
                nc.vector.scalar_tensor_tensor(
                    gt[:], rawh[:], nw[:], sgh[:],
                    op0=OP.mult, op1=OP.mult)
                nc.sync.dma_start(gstage[h][:], gt[:])

        # ================= Phase C: o_proj =================
        with ExitStack() as ctx:
            wop = ctx.enter_context(tc.tile_pool(name="wo", bufs=1))
            gpool = ctx.enter_context(tc.tile_pool(name="gres", bufs=1))
            wot, gres = [], []
            for h in range(HPC):
                t = wop.tile([P, HID], f16, tag=f"wo{h}")
                nc.sync.dma_start(t[:], woT[h * P:(h + 1) * P, :])
                wot.append(t)
                g = gpool.tile([P, L], f16, tag=f"gr{h}")
                nc.sync.dma_start(g[:], gstage[h][:])
                gres.append(g)
            opp = ctx.enter_context(
                tc.tile_pool(name="oproj_psum", bufs=2, space="PSUM"))
            oev = ctx.enter_context(tc.tile_pool(name="oev", bufs=3))
            for mc in range(L // P):
                ops = opp.tile([P, HID], f32, tag="op")
                for h in range(HPC):
                    for s in range(HID // QT):
                        nc.tensor.matmul(
                            ops[:, s * QT:(s + 1) * QT],
                            gres[h][:, mc * P:(mc + 1) * P],
                            wot[h][:, s * QT:(s + 1) * QT],
                            start=(h == 0),
                            stop=(h == HPC - 1),
                        )
                ot = oev.tile([P, HID], f32, tag="ot")
                nc.scalar.copy(ot[:], ops[:])
                nc.sync.dma_start(out_partial[mc * P:(mc + 1) * P, :], ot[:])

    return nc


_STATE = {}


def _get_state():
    if _STATE:
        return _STATE
    import jax
    import jax.numpy as jnp
    from jax.sharding import Mesh, PartitionSpec, NamedSharding
    from jax.experimental.shard_map import shard_map
    import concourse.bacc as bacc
    import concourse.mybir as mybir
    import concourse.tile as tile
    from concourse.bass2jax import (
        _bass_exec_p, partition_id_tensor, install_neuronx_cc_hook)

    install_neuronx_cc_hook()
    nc = bacc.Bacc("TRN2", target_bir_lowering=False, debug=False)
    _build(nc, mybir, tile)
    nc.compile()

    partition_name = (nc.partition_id_tensor.name
                      if nc.partition_id_tensor is not None else None)
    in_names, out_names, out_avals = [], [], []
    for alloc in nc.m.functions[0].allocations:
        if not isinstance(alloc, mybir.MemoryLocationSet):
            continue
        name = alloc.memorylocations[0].name
        if alloc.kind == "ExternalInput":
            if name != partition_name:
                in_names.append(name)
        elif alloc.kind == "ExternalOutput":
            shape = tuple(alloc.tensor_shape)
            dtype = mybir.dt.np(alloc.dtype)
            out_names.append(name)
            out_avals.append(jax.core.ShapedArray(shape, dtype))
    n_params = len(in_names)
    n_outs = len(out_names)
    all_in_names = list(in_names) + list(out_names)
    if partition_name is not None:
        all_in_names.append(partition_name)

    devs = jax.devices()[:NCORES]
    mesh = Mesh(np.asarray(devs), ("core",))
    Pc = PartitionSpec("core")
    shc = NamedSharding(mesh, Pc)

    def bass_body(*args):
        operands = list(args)
        if partition_name is not None:
            operands.append(partition_id_tensor())
        outs = _bass_exec_p.bind(
            *operands,
            out_avals=tuple(out_avals),
            in_names=tuple(all_in_names),
            out_names=tuple(out_names),
            lowering_input_output_aliases=(),
            sim_require_finite=True,
            sim_require_nnan=True,
            nc=nc,
        )
        return tuple(outs)

    bass_jit = jax.jit(
        shard_map(bass_body, mesh=mesh,
                  in_specs=(Pc,) * (n_params + n_outs),
                  out_specs=(Pc,) * n_outs, check_rep=False),
        donate_argnums=tuple(range(n_params, n_params + n_outs)),
        keep_unused=True,
    )

    cosq_c, ssinq_c, cosk_c, ssink_c = _COSQ, _SSINQ, _COSK, _SSINK
    masks_c = _MASKS

    def gather_body(x_sh, w_sh, wo_sh, nw_sh):
        xb = jax.lax.all_gather(
            x_sh, "core", axis_index_groups=[[0, 1, 2, 3], [4, 5, 6, 7]],
            tiled=True)                                   # [L, HID] f16
        xTg = xb.T                                        # [HID, L]
        wfull = jax.lax.all_gather(
            w_sh, "core", axis_index_groups=[[0, 4], [1, 5], [2, 6], [3, 7]],
            tiled=True)                                   # [NCH*P, HID] f16
        wTbg = wfull.T.reshape(KC, P, NCH, P).transpose(0, 2, 1, 3)
        woTg = jax.lax.all_gather(
            wo_sh, "core", axis_index_groups=[[0, 4], [1, 5], [2, 6], [3, 7]],
            tiled=True)                                   # [NDIM, HID] f16
        ones = jnp.ones((P, P), jnp.float16)
        oneshd = jnp.full((P, P), 1.0 / HD, jnp.bfloat16)
        ident = jnp.eye(P, dtype=jnp.float16)
        zeros = jnp.zeros((L, HID), jnp.float32)
        return (xTg, wTbg, woTg, jnp.asarray(cosq_c), jnp.asarray(ssinq_c),
                jnp.asarray(cosk_c), jnp.asarray(ssink_c),
                jnp.asarray(masks_c), ones, oneshd, ident, nw_sh, zeros)

    gather_jit = jax.jit(
        shard_map(gather_body, mesh=mesh, in_specs=(Pc,) * 4,
                  out_specs=(Pc,) * 13, check_rep=False))

    def psum_body(p):
        s = jax.lax.psum(p, "core",
                         axis_index_groups=[[0, 1, 2, 3], [4, 5, 6, 7]])
        return s.astype(jnp.float16)

    psum_jit = jax.jit(
        shard_map(psum_body, mesh=mesh, in_specs=Pc, out_specs=Pc,
                  check_rep=False))

    _STATE.update(
        jax=jax, mesh=mesh, shc=shc, in_names=in_names, out_names=out_names,
        gather_jit=gather_jit, bass_jit=bass_jit, psum_jit=psum_jit)
    return _STATE


_GNAMES = ["xT", "wTb", "woT", "cosq", "ssinq", "cosk", "ssink",
           "masks_t", "ones_t", "oneshd_t", "ident_t", "nw_t", "__zeros__"]


def kernel(hidden_states, wq, wk, wv, wg, wo, norm_w, _trace=False):
    st = _get_state()
    jax = st["jax"]

    # host prep: fp16 casts laid out so each byte crosses the wire once
    x_st = np.asarray(hidden_states, np.float32).reshape(
        NCORES * QT, HID).astype(np.float16)
    xd = jax.device_put(x_st, st["shc"])

    wqh = np.asarray(wq, np.float32).astype(np.float16)
    wkh = np.asarray(wk, np.float32).astype(np.float16)
    wvh = np.asarray(wv, np.float32).astype(np.float16)
    wgh = np.asarray(wg, np.float32).astype(np.float16)
    w_st = np.empty((8 * 1024, HID), np.float16)
    for g in range(4):
        hs = slice(NDIM * g, NDIM * (g + 1))
        w_st[1024 * g:1024 * g + 512] = wqh[hs]
        w_st[1024 * g + 512:1024 * (g + 1)] = wkh[hs]
        w_st[4096 + 1024 * g:4096 + 1024 * g + 512] = wvh[hs]
        w_st[4096 + 1024 * g + 512:4096 + 1024 * (g + 1)] = wgh[hs]
    wd = jax.device_put(w_st, st["shc"])

    woT16 = np.ascontiguousarray(np.asarray(wo, np.float32)
                                 .astype(np.float16).T)
    wo_st = np.ascontiguousarray(
        woT16.reshape(4, 2, 256, HID).transpose(1, 0, 2, 3).reshape(2048, HID))
    wod = jax.device_put(wo_st, st["shc"])

    nw_g = np.tile(np.asarray(norm_w, np.float32).reshape(P, 1), (NCORES, 1))
    nwd = jax.device_put(nw_g, st["shc"])

    g_outs = st["gather_jit"](xd, wd, wod, nwd)
    gmap = dict(zip(_GNAMES, g_outs))
    bass_args = [gmap[n] for n in st["in_names"]] + [gmap["__zeros__"]]
    (partial,) = st["bass_jit"](*bass_args)
    summed = st["psum_jit"](partial)

    shard_data = {}
    for shard in summed.addressable_shards:
        start = shard.index[0].start or 0
        if start == 0:
            shard_data[0] = shard.data
        elif start == 4 * L:
            shard_data[1] = shard.data
    from concurrent.futures import ThreadPoolExecutor
    with ThreadPoolExecutor(2) as ex:
        f0 = ex.submit(np.asarray, shard_data[0])
        f1 = ex.submit(np.asarray, shard_data[1])
        out = np.empty((B, L, HID), np.float32)
        out[0] = f0.result()
        out[1] = f1.result()
    return out
